# revision 12
# baseline (speedup 1.0000x reference)
"""MetaSR super-resolution Trainium2 kernel (bf16 v11).

Structure exploited: out_h=out_w=256 with H=W=64 LR grid means the scale
factor is exactly 4, so the nearest-neighbor gather index is iy=oy//4,
ix=ox//4 and the per-query MLP input collapses to 16 distinct subpixel
phases [dy/4, dx/4, 0.25].  The whole model becomes:

  1. h    = relu(mlp_in @ w1 + b1)              [16, 256]
  2. predw = h @ w2 + b2                        [16, 576, 3]
  3. rgb[o, 4*iy+dy, 4*ix+dx] =
       sum_{c,ki,kj} feat[c, iy+ki-1, ix+kj-1] * predw[(dy,dx), c*9+ki*3+kj, o]
     i.e. a 3x3 conv with 64 in / 48 out channels + pixel shuffle.

Sharding: data-parallel over LR rows (8 rows per core, 10-row halo band),
weights replicated; steps 1+2 are recomputed on every core (tiny).

The conv contraction (K = 9 taps x 64 ch = 576) is chunked K=128 by pairing
taps; each core holds the zero-padded band twice in a 128-partition tile at
free-dim offsets differing by the tap-pair shift delta (see CHUNK_SPECS).

All tensors bf16 (simulated end-to-end rel err 4.0e-3, gate 2e-2): 1-pass
PE matmuls, ~1.25MB/core input DMA.

DMA model (from NTFF traces): the 16 SDMA engines drain the two HWDGE
rings' packets in a near-serial global round-robin, so blob ARRIVAL order
follows ISSUE order across both rings and total stream time is the sum
over blobs (~280-430 GB/s).  Completion semaphores fire ~0.7us after a
blob's last row only when each engine's share (8 rows + completion desc)
fits one packet — keep rows <= 1536B.  Hence: 8 blobs <= 196KB, issued
alternating rings in exact first-use order, with conv chunks reordered
[0,2,3,1,4] so the band2-dependent chunks run last.
"""

import numpy as np
import ml_dtypes

try:
    import concourse.bass as bass
except ImportError:  # fall back to the repo checkout
    import sys
    sys.path.insert(0, "/opt/trn_rl_repo")
    import concourse.bass as bass
import concourse.mybir as mybir
import concourse.tile as tile
from concourse import bacc
from concourse.bass_utils import run_bass_kernel_spmd

F32 = mybir.dt.float32
BF16 = mybir.dt.bfloat16
N_CORES = 8
ROWS_PER_CORE = 8          # LR rows per core
BAND_ROWS = ROWS_PER_CORE + 2
NPOS = ROWS_PER_CORE * 64  # 512 LR positions per core

# Tap order for K-chunking.  Taps t = ki*3+kj have band shift ki*66+kj:
#   t:      0   1   2   3    4    5    6    7    8
#   shift:  0   1   2   66   67   68   132  133  134
# chunk0: [t0; t1] band1 off 1 | chunk1: [t3; t2] band2 off 66
# chunk2: [t4; t5] band1 off 68 | chunk3: [t6; t7] band1 off 133
# chunk4: [t8] band2 off 134 (K=64)
TAP_ORDER = [0, 1, 3, 2, 4, 5, 6, 7, 8]
CHUNK_SPECS = [  # (band_tile_idx, rhs_offset, K)
    (0, 1, 128),
    (1, 66, 128),
    (0, 68, 128),
    (0, 133, 128),
    (1, 134, 64),
]
CHUNK_ORDER = [0, 2, 3, 1, 4]  # band1 chunks first; band2 + K=64 last

# small [128, 306] bf16: b1b2 [128,17] f32 (bf16 cols 0:34),
# w1 [3,256] bf16 (rows 0-2, cols 34:290), mlpin [3,16] bf16 (cols 290:306)
OFF_W1 = 34
OFF_MLP = 290
OFF_B2BLK = 306   # 5 x [128, 48] bf16 bias blocks (b2 broadcast over phases)
COLS_SMALL = 546
COLS_BAND = 660   # unique band data [64, 660], shipped row-duplicated [128, 660]
COLS_BAND1 = 661
COLS_BAND2 = 724
COLS_M = 768      # w2 m-block (m<4): 6 sub-blocks (o*2+hc) x [128, 128]
COLS_M4 = 384     # m=4 block: 6 x [128, 64] (top half only)

N_WARMUP_MM = 3

_CACHE = {}


def _build_program():
    """Build + compile the single-core Bass program (same for all cores)."""
    nc = bacc.Bacc("TRN2", target_bir_lowering=False, debug=False)

    small_d = nc.dram_tensor("small", [128, COLS_SMALL], BF16, kind="ExternalInput")
    band1_d = nc.dram_tensor("band1", [128, COLS_BAND1], BF16, kind="ExternalInput")
    band2_d = nc.dram_tensor("band2", [128, COLS_BAND2], BF16, kind="ExternalInput")
    mds = [
        nc.dram_tensor(f"m{m}", [128, COLS_M if m < 4 else COLS_M4], BF16,
                       kind="ExternalInput")
        for m in range(5)
    ]
    out48 = nc.dram_tensor("out48", [48, NPOS], BF16, kind="ExternalOutput")

    with tile.TileContext(nc) as tc:
        with (
            tc.tile_pool(name="blobs", bufs=1) as blobs,
            tc.tile_pool(name="work", bufs=1) as work,
            tc.tile_pool(name="wpool", bufs=5) as wpool,
            tc.tile_pool(name="ps_small", bufs=2, space="PSUM") as ps_small,
            tc.tile_pool(name="ps_w", bufs=3, space="PSUM") as ps_w,
            tc.tile_pool(name="ps_rgb", bufs=1, space="PSUM") as ps_rgb,
        ):
            # Issue alternating rings in first-use order; the SDMA round-robin
            # then streams them in roughly this sequence.
            # Issue alternating rings in first-use order; the SDMA round-robin
            # then streams them in roughly this sequence.
            small = blobs.tile([128, COLS_SMALL], BF16, tag="small")
            nc.sync.dma_start(small[:, :], small_d[:, :])
            m0 = blobs.tile([128, COLS_M], BF16, tag="m0")
            nc.scalar.dma_start(m0[:, :], mds[0][:, :])
            m2 = blobs.tile([128, COLS_M], BF16, tag="m2")
            nc.sync.dma_start(m2[:, :], mds[2][:, :])
            band1 = blobs.tile([128, COLS_BAND1], BF16, tag="band1")
            nc.scalar.dma_start(band1[:, :], band1_d[:, :])
            m3 = blobs.tile([128, COLS_M], BF16, tag="m3")
            nc.sync.dma_start(m3[:, :], mds[3][:, :])
            m1 = blobs.tile([128, COLS_M], BF16, tag="m1")
            nc.scalar.dma_start(m1[:, :], mds[1][:, :])
            band2 = blobs.tile([128, COLS_BAND2], BF16, tag="band2")
            nc.sync.dma_start(band2[:, :], band2_d[:, :])
            m4 = blobs.tile([128, COLS_M4], BF16, tag="m4")
            nc.scalar.dma_start(m4[:, :], mds[4][:, :])

            b1b2 = small.bitcast(F32)[:, 0:17]
            w1_sb = small[0:3, OFF_W1:OFF_W1 + 256]
            mlp_sb = small[0:3, OFF_MLP:OFF_MLP + 16]
            band_tiles = [band1, band2]
            mtiles = [m0, m1, m2, m3, m4]

            def w2_slice(m, o, hc, msize):
                return mtiles[m][:, (o * 2 + hc) * msize:(o * 2 + hc + 1) * msize]

            # ---- PE warm-up: dummy zero matmuls into rgb_ps while DMAs run.
            # conv chunk 0 below uses start=True, which resets the PSUM
            # accumulation, so these contribute nothing to the result.
            rgb_ps = ps_rgb.tile([48, NPOS], F32, tag="rgb")
            warm = work.tile([128, 512], BF16, tag="warm")
            nc.vector.memset(warm[:, :], 0.0)
            for _ in range(N_WARMUP_MM):
                nc.tensor.matmul(
                    rgb_ps[:, :], warm[:, 0:48], warm[:, 0:NPOS],
                    start=True, stop=True,
                )

            # ---- MLP layer 1: h_actT [256, 16] in two 128-chunks ----
            h_sb = work.tile([128, 32], BF16, tag="hact")
            for hc in range(2):
                ph = ps_small.tile([128, 16], F32, tag="ph")
                nc.tensor.matmul(
                    ph[:, :], w1_sb[:, hc * 128:(hc + 1) * 128], mlp_sb[:, :],
                    start=True, stop=True,
                )
                # relu(x + b1) = max(x + b1, 0) in one DVE op
                nc.vector.tensor_scalar(
                    h_sb[:, hc * 16:(hc + 1) * 16], ph[:, :],
                    b1b2[:, hc:hc + 1], 0.0,
                    mybir.AluOpType.add, mybir.AluOpType.max,
                )

            # ---- per K-chunk: W assembly (MLP layer 2) + conv half 0 ----
            # The two 256-col conv halves run as two sequential well-formed
            # PSUM accumulation groups; F=256 matmuls pipeline 2-deep on the
            # PE (~210ns effective vs ~600ns serial for F=512).
            w_tiles = {}
            for i, m in enumerate(CHUNK_ORDER):
                bidx, roff, K = CHUNK_SPECS[m]
                msize = K
                w_sb = wpool.tile([128, 48], BF16, tag="W")
                w_tiles[m] = w_sb
                pw = ps_w.tile([128, 48], F32, tag="pw")
                for o in range(3):
                    for hc in range(2):
                        nc.tensor.matmul(
                            pw[:msize, o * 16:(o + 1) * 16],
                            w2_slice(m, o, hc, msize),
                            h_sb[:, hc * 16:(hc + 1) * 16],
                            start=(hc == 0), stop=(hc == 1),
                        )
                nc.vector.tensor_add(
                    w_sb[:msize, :], pw[:msize, :],
                    small[:msize, OFF_B2BLK + m * 48:OFF_B2BLK + (m + 1) * 48],
                )
                bt = band_tiles[bidx]
                rhs = bt[0:K, roff:roff + 264].rearrange(
                    "p (r c) -> p r c", c=66)[:, :, 0:64]
                nc.tensor.matmul(
                    rgb_ps[:, 0:256], w_sb[:msize, :], rhs,
                    start=(i == 0), stop=(i == len(CHUNK_ORDER) - 1),
                )

            # ---- conv half 1 ----
            for i, m in enumerate(CHUNK_ORDER):
                bidx, roff, K = CHUNK_SPECS[m]
                bt = band_tiles[bidx]
                rhs = bt[0:K, roff + 264:roff + 528].rearrange(
                    "p (r c) -> p r c", c=66)[:, :, 0:64]
                nc.tensor.matmul(
                    rgb_ps[:, 256:512], w_tiles[m][:K, :], rhs,
                    start=(i == 0), stop=(i == len(CHUNK_ORDER) - 1),
                )

            # ---- write out ----
            out_sb = work.tile([48, NPOS], BF16, tag="out")
            nc.vector.tensor_copy(out_sb[:, :], rgb_ps[:, :])
            nc.sync.dma_start(out48[:, :], out_sb[:, :])

    nc.compile()
    return nc


def _bf16(x):
    return np.asarray(x, dtype=np.float32).astype(ml_dtypes.bfloat16)


def _host_prep(feat, w1, b1, w2, b2):
    """Pack shared blobs + per-core band blobs (bf16)."""
    feat = np.ascontiguousarray(np.asarray(feat, dtype=np.float32))[0]  # [64,64,64]
    w1 = np.asarray(w1, dtype=np.float32)
    b1 = np.asarray(b1, dtype=np.float32)
    w2 = np.asarray(w2, dtype=np.float32)
    b2 = np.asarray(b2, dtype=np.float32)

    dydx = np.arange(16)
    mlpin = np.stack(
        [dydx // 4 / 4.0, dydx % 4 / 4.0, np.full(16, 0.25)], axis=0
    ).astype(np.float32)  # [3, 16]

    # tap-major permutations of w2/b2
    w2r = w2.reshape(256, 64, 9, 3)  # [h, c, t, o]
    w2p = np.empty((3, 256, 576), dtype=np.float32)
    b2r = b2.reshape(64, 9, 3)       # [c, t, o]
    b2p = np.empty((3, 576), dtype=np.float32)
    for blk, t in enumerate(TAP_ORDER):
        w2p[:, :, blk * 64:(blk + 1) * 64] = w2r[:, :, t, :].transpose(2, 0, 1)
        b2p[:, blk * 64:(blk + 1) * 64] = b2r[:, t, :].T
    w2p = _bf16(w2p)

    b1b2 = np.zeros((128, 17), dtype=np.float32)
    b1b2[:, 0] = b1[0:128]
    b1b2[:, 1] = b1[128:256]
    for o in range(3):
        for m in range(5):
            msize = 128 if m < 4 else 64
            b1b2[:msize, 2 + o * 5 + m] = b2p[o, 128 * m:128 * m + msize]

    small = np.zeros((128, COLS_SMALL), dtype=ml_dtypes.bfloat16)
    small.view(np.uint16)[:, 0:34] = b1b2.view(np.uint16)
    small[0:3, OFF_W1:OFF_W1 + 256] = _bf16(w1)
    small[0:3, OFF_MLP:OFF_MLP + 16] = _bf16(mlpin)
    for m in range(5):
        msize = 128 if m < 4 else 64
        for o in range(3):
            small[:msize, OFF_B2BLK + m * 48 + o * 16:
                  OFF_B2BLK + m * 48 + (o + 1) * 16] = \
                _bf16(b2p[o, 128 * m:128 * m + msize])[:, None]

    mblobs = []
    for m in range(5):
        msize = 128 if m < 4 else 64
        blob = np.empty((128, 6 * msize), dtype=ml_dtypes.bfloat16)
        for o in range(3):
            for hc in range(2):
                base = (o * 2 + hc) * msize
                blob[:, base:base + msize] = \
                    w2p[o, hc * 128:(hc + 1) * 128, 128 * m:128 * m + msize]
        mblobs.append(blob)

    featp = np.zeros((64, 66, 66), dtype=np.float32)
    featp[:, 1:65, 1:65] = feat
    featp = _bf16(featp)

    bands1, bands2 = [], []
    for core in range(N_CORES):
        r0 = core * ROWS_PER_CORE
        band = featp[:, r0:r0 + BAND_ROWS, :].reshape(64, BAND_ROWS * 66)
        bb1 = np.zeros((128, COLS_BAND1), dtype=ml_dtypes.bfloat16)
        bb1[0:64, 1:661] = band
        bb1[64:128, 0:660] = band
        bb2 = np.zeros((128, COLS_BAND2), dtype=ml_dtypes.bfloat16)
        bb2[0:64, 0:660] = band
        bb2[64:128, 64:724] = band
        bands1.append(bb1)
        bands2.append(bb2)
    return small, mblobs, bands1, bands2


def _assemble(per_core_out48):
    """[8 x [48, 512]] -> [1, 3, 256, 256]."""
    full = np.stack([np.asarray(x, dtype=np.float32) for x in per_core_out48])
    full = full.reshape(8, 3, 4, 4, 8, 64)               # [core, o, dy, dx, r, x]
    rgb = full.transpose(1, 0, 4, 2, 5, 3).reshape(3, 256, 256)
    return np.ascontiguousarray(rgb)[None]


def get_program():
    if "nc" not in _CACHE:
        _CACHE["nc"] = _build_program()
    return _CACHE["nc"]


def run(feat, w1, b1, w2, b2, out_h, out_w, trace=False, **spmd_kwargs):
    assert int(out_h) == 256 and int(out_w) == 256
    nc = get_program()
    small, mblobs, bands1, bands2 = _host_prep(feat, w1, b1, w2, b2)
    in_maps = [
        {"small": small, "band1": bands1[core], "band2": bands2[core],
         **{f"m{m}": mblobs[m] for m in range(5)}}
        for core in range(N_CORES)
    ]
    res = run_bass_kernel_spmd(
        nc, in_maps, core_ids=list(range(N_CORES)), trace=trace, **spmd_kwargs
    )
    out = _assemble([res.results[core]["out48"] for core in range(N_CORES)])
    return out, res


def kernel(feat, w1, b1, w2, b2, out_h, out_w):
    out, _ = run(feat, w1, b1, w2, b2, out_h, out_w, trace=False)
    return out


# revision 13
# speedup vs baseline: 1.0779x; 1.0779x over previous
"""MetaSR super-resolution Trainium2 kernel (bf16 v5-final).

Structure exploited: out_h=out_w=256 with H=W=64 LR grid means the scale
factor is exactly 4, so the nearest-neighbor gather index is iy=oy//4,
ix=ox//4 and the per-query MLP input collapses to 16 distinct subpixel
phases [dy/4, dx/4, 0.25].  The whole model becomes:

  1. h    = relu(mlp_in @ w1 + b1)              [16, 256]
  2. predw = h @ w2 + b2                        [16, 576, 3]
  3. rgb[o, 4*iy+dy, 4*ix+dx] =
       sum_{c,ki,kj} feat[c, iy+ki-1, ix+kj-1] * predw[(dy,dx), c*9+ki*3+kj, o]
     i.e. a 3x3 conv with 64 in / 48 out channels + pixel shuffle.

Sharding: data-parallel over LR rows (8 rows per core, 10-row halo band),
weights replicated; steps 1+2 are recomputed on every core (tiny).

The conv contraction (K = 9 taps x 64 ch = 576) is chunked K=128 by pairing
taps; each core holds the zero-padded band twice in a 128-partition tile at
free-dim offsets differing by the tap-pair shift delta (see CHUNK_SPECS).

All tensors bf16 (simulated end-to-end rel err 4.0e-3, gate 2e-2): 1-pass
PE matmuls, ~1.25MB/core input DMA.

DMA model (from NTFF traces): the 16 SDMA engines drain the two HWDGE
rings' packets in a near-serial global round-robin, so blob ARRIVAL order
follows ISSUE order across both rings and total stream time is the sum
over blobs (~280-430 GB/s).  Completion semaphores fire ~0.7us after a
blob's last row only when each engine's share (8 rows + completion desc)
fits one packet — keep rows <= 1536B.  Hence: 8 blobs <= 196KB, issued
alternating rings in exact first-use order, with conv chunks reordered
[0,2,3,1,4] so the band2-dependent chunks run last.
"""

import numpy as np
import ml_dtypes

try:
    import concourse.bass as bass
except ImportError:  # fall back to the repo checkout
    import sys
    sys.path.insert(0, "/opt/trn_rl_repo")
    import concourse.bass as bass
import concourse.mybir as mybir
import concourse.tile as tile
from concourse import bacc
from concourse.bass_utils import run_bass_kernel_spmd

F32 = mybir.dt.float32
BF16 = mybir.dt.bfloat16
N_CORES = 8
ROWS_PER_CORE = 8          # LR rows per core
BAND_ROWS = ROWS_PER_CORE + 2
NPOS = ROWS_PER_CORE * 64  # 512 LR positions per core

# Tap order for K-chunking.  Taps t = ki*3+kj have band shift ki*66+kj:
#   t:      0   1   2   3    4    5    6    7    8
#   shift:  0   1   2   66   67   68   132  133  134
# chunk0: [t0; t1] band1 off 1 | chunk1: [t3; t2] band2 off 66
# chunk2: [t4; t5] band1 off 68 | chunk3: [t6; t7] band1 off 133
# chunk4: [t8] band2 off 134 (K=64)
TAP_ORDER = [0, 1, 3, 2, 4, 5, 6, 7, 8]
CHUNK_SPECS = [  # (band_tile_idx, rhs_offset, K)
    (0, 1, 128),
    (1, 66, 128),
    (0, 68, 128),
    (0, 133, 128),
    (1, 134, 64),
]
CHUNK_ORDER = [0, 2, 3, 1, 4]  # band1 chunks first; band2 + K=64 last

# small [128, 306] bf16: b1b2 [128,17] f32 (bf16 cols 0:34),
# w1 [3,256] bf16 (rows 0-2, cols 34:290), mlpin [3,16] bf16 (cols 290:306)
OFF_W1 = 34
OFF_MLP = 290
COLS_SMALL = 306
COLS_BAND = 660   # unique band data [64, 660], shipped row-duplicated [128, 660]
COLS_BAND1 = 661
COLS_BAND2 = 724
COLS_M = 768      # w2 m-block (m<4): 6 sub-blocks (o*2+hc) x [128, 128]
COLS_M4 = 384     # m=4 block: 6 x [128, 64] (top half only)

N_WARMUP_MM = 3

_CACHE = {}


def _build_program():
    """Build + compile the single-core Bass program (same for all cores)."""
    nc = bacc.Bacc("TRN2", target_bir_lowering=False, debug=False)

    small_d = nc.dram_tensor("small", [128, COLS_SMALL], BF16, kind="ExternalInput")
    band1_d = nc.dram_tensor("band1", [128, COLS_BAND1], BF16, kind="ExternalInput")
    band2_d = nc.dram_tensor("band2", [128, COLS_BAND2], BF16, kind="ExternalInput")
    mds = [
        nc.dram_tensor(f"m{m}", [128, COLS_M if m < 4 else COLS_M4], BF16,
                       kind="ExternalInput")
        for m in range(5)
    ]
    out48 = nc.dram_tensor("out48", [48, NPOS], BF16, kind="ExternalOutput")

    with tile.TileContext(nc) as tc:
        with (
            tc.tile_pool(name="blobs", bufs=1) as blobs,
            tc.tile_pool(name="work", bufs=1) as work,
            tc.tile_pool(name="wpool", bufs=5) as wpool,
            tc.tile_pool(name="ps_small", bufs=2, space="PSUM") as ps_small,
            tc.tile_pool(name="ps_w", bufs=5, space="PSUM") as ps_w,
            tc.tile_pool(name="ps_rgb", bufs=1, space="PSUM") as ps_rgb,
        ):
            # Issue alternating rings in first-use order; the SDMA round-robin
            # then streams them in roughly this sequence.
            # Issue alternating rings in first-use order; the SDMA round-robin
            # then streams them in roughly this sequence.
            small = blobs.tile([128, COLS_SMALL], BF16, tag="small")
            nc.sync.dma_start(small[:, :], small_d[:, :])
            m0 = blobs.tile([128, COLS_M], BF16, tag="m0")
            nc.scalar.dma_start(m0[:, :], mds[0][:, :])
            m2 = blobs.tile([128, COLS_M], BF16, tag="m2")
            nc.sync.dma_start(m2[:, :], mds[2][:, :])
            band1 = blobs.tile([128, COLS_BAND1], BF16, tag="band1")
            nc.scalar.dma_start(band1[:, :], band1_d[:, :])
            m3 = blobs.tile([128, COLS_M], BF16, tag="m3")
            nc.sync.dma_start(m3[:, :], mds[3][:, :])
            m1 = blobs.tile([128, COLS_M], BF16, tag="m1")
            nc.scalar.dma_start(m1[:, :], mds[1][:, :])
            band2 = blobs.tile([128, COLS_BAND2], BF16, tag="band2")
            nc.sync.dma_start(band2[:, :], band2_d[:, :])
            m4 = blobs.tile([128, COLS_M4], BF16, tag="m4")
            nc.scalar.dma_start(m4[:, :], mds[4][:, :])

            b1b2 = small.bitcast(F32)[:, 0:17]
            w1_sb = small[0:3, OFF_W1:OFF_W1 + 256]
            mlp_sb = small[0:3, OFF_MLP:OFF_MLP + 16]
            band_tiles = [band1, band2]
            mtiles = [m0, m1, m2, m3, m4]

            def w2_slice(m, o, hc, msize):
                return mtiles[m][:, (o * 2 + hc) * msize:(o * 2 + hc + 1) * msize]

            # ---- PE warm-up: dummy zero matmuls into rgb_ps while DMAs run.
            # conv chunk 0 below uses start=True, which resets the PSUM
            # accumulation, so these contribute nothing to the result.
            rgb_ps = ps_rgb.tile([48, NPOS], F32, tag="rgb")
            warm = work.tile([128, 512], BF16, tag="warm")
            nc.vector.memset(warm[:, :], 0.0)
            for _ in range(N_WARMUP_MM):
                nc.tensor.matmul(
                    rgb_ps[:, :], warm[:, 0:48], warm[:, 0:NPOS],
                    start=True, stop=True,
                )

            # ---- MLP layer 1: h_actT [256, 16] in two 128-chunks ----
            h_sb = work.tile([128, 32], BF16, tag="hact")
            for hc in range(2):
                ph = ps_small.tile([128, 16], F32, tag="ph")
                nc.tensor.matmul(
                    ph[:, :], w1_sb[:, hc * 128:(hc + 1) * 128], mlp_sb[:, :],
                    start=True, stop=True,
                )
                # relu(x + b1) = max(x + b1, 0) in one DVE op
                nc.vector.tensor_scalar(
                    h_sb[:, hc * 16:(hc + 1) * 16], ph[:, :],
                    b1b2[:, hc:hc + 1], 0.0,
                    mybir.AluOpType.add, mybir.AluOpType.max,
                )

            # ---- per K-chunk: W assembly (MLP layer 2) + conv matmul ----
            w_tiles = {}
            for i, m in enumerate(CHUNK_ORDER):
                bidx, roff, K = CHUNK_SPECS[m]
                msize = K
                w_sb = wpool.tile([128, 48], BF16, tag="W")
                w_tiles[m] = w_sb
                for o in range(3):
                    pw = ps_w.tile([128, 16], F32, tag="pw")
                    for hc in range(2):
                        nc.tensor.matmul(
                            pw[:msize, :],
                            w2_slice(m, o, hc, msize),
                            h_sb[:, hc * 16:(hc + 1) * 16],
                            start=(hc == 0), stop=(hc == 1),
                        )
                    nc.vector.tensor_scalar_add(
                        w_sb[:msize, o * 16:(o + 1) * 16], pw[:msize, :],
                        b1b2[:msize, 2 + o * 5 + m:3 + o * 5 + m],
                    )
                bt = band_tiles[bidx]
                rhs = bt[0:K, roff:roff + 528].rearrange(
                    "p (r c) -> p r c", c=66
                )[:, :, 0:64]
                nc.tensor.matmul(
                    rgb_ps[:, :], w_sb[:msize, :], rhs,
                    start=(i == 0), stop=(i == len(CHUNK_ORDER) - 1),
                )

            # ---- write out ----
            out_sb = work.tile([48, NPOS], BF16, tag="out")
            nc.vector.tensor_copy(out_sb[:, :], rgb_ps[:, :])
            nc.sync.dma_start(out48[:, :], out_sb[:, :])

    nc.compile()
    return nc


def _bf16(x):
    return np.asarray(x, dtype=np.float32).astype(ml_dtypes.bfloat16)


def _host_prep(feat, w1, b1, w2, b2):
    """Pack shared blobs + per-core band blobs (bf16)."""
    feat = np.ascontiguousarray(np.asarray(feat, dtype=np.float32))[0]  # [64,64,64]
    w1 = np.asarray(w1, dtype=np.float32)
    b1 = np.asarray(b1, dtype=np.float32)
    w2 = np.asarray(w2, dtype=np.float32)
    b2 = np.asarray(b2, dtype=np.float32)

    dydx = np.arange(16)
    mlpin = np.stack(
        [dydx // 4 / 4.0, dydx % 4 / 4.0, np.full(16, 0.25)], axis=0
    ).astype(np.float32)  # [3, 16]

    # tap-major permutations of w2/b2
    w2r = w2.reshape(256, 64, 9, 3)  # [h, c, t, o]
    w2p = np.empty((3, 256, 576), dtype=np.float32)
    b2r = b2.reshape(64, 9, 3)       # [c, t, o]
    b2p = np.empty((3, 576), dtype=np.float32)
    for blk, t in enumerate(TAP_ORDER):
        w2p[:, :, blk * 64:(blk + 1) * 64] = w2r[:, :, t, :].transpose(2, 0, 1)
        b2p[:, blk * 64:(blk + 1) * 64] = b2r[:, t, :].T
    w2p = _bf16(w2p)

    b1b2 = np.zeros((128, 17), dtype=np.float32)
    b1b2[:, 0] = b1[0:128]
    b1b2[:, 1] = b1[128:256]
    for o in range(3):
        for m in range(5):
            msize = 128 if m < 4 else 64
            b1b2[:msize, 2 + o * 5 + m] = b2p[o, 128 * m:128 * m + msize]

    small = np.zeros((128, COLS_SMALL), dtype=ml_dtypes.bfloat16)
    small.view(np.uint16)[:, 0:34] = b1b2.view(np.uint16)
    small[0:3, OFF_W1:OFF_W1 + 256] = _bf16(w1)
    small[0:3, OFF_MLP:OFF_MLP + 16] = _bf16(mlpin)

    mblobs = []
    for m in range(5):
        msize = 128 if m < 4 else 64
        blob = np.empty((128, 6 * msize), dtype=ml_dtypes.bfloat16)
        for o in range(3):
            for hc in range(2):
                base = (o * 2 + hc) * msize
                blob[:, base:base + msize] = \
                    w2p[o, hc * 128:(hc + 1) * 128, 128 * m:128 * m + msize]
        mblobs.append(blob)

    featp = np.zeros((64, 66, 66), dtype=np.float32)
    featp[:, 1:65, 1:65] = feat
    featp = _bf16(featp)

    bands1, bands2 = [], []
    for core in range(N_CORES):
        r0 = core * ROWS_PER_CORE
        band = featp[:, r0:r0 + BAND_ROWS, :].reshape(64, BAND_ROWS * 66)
        bb1 = np.zeros((128, COLS_BAND1), dtype=ml_dtypes.bfloat16)
        bb1[0:64, 1:661] = band
        bb1[64:128, 0:660] = band
        bb2 = np.zeros((128, COLS_BAND2), dtype=ml_dtypes.bfloat16)
        bb2[0:64, 0:660] = band
        bb2[64:128, 64:724] = band
        bands1.append(bb1)
        bands2.append(bb2)
    return small, mblobs, bands1, bands2


def _assemble(per_core_out48):
    """[8 x [48, 512]] -> [1, 3, 256, 256]."""
    full = np.stack([np.asarray(x, dtype=np.float32) for x in per_core_out48])
    full = full.reshape(8, 3, 4, 4, 8, 64)               # [core, o, dy, dx, r, x]
    rgb = full.transpose(1, 0, 4, 2, 5, 3).reshape(3, 256, 256)
    return np.ascontiguousarray(rgb)[None]


def get_program():
    if "nc" not in _CACHE:
        _CACHE["nc"] = _build_program()
    return _CACHE["nc"]


def run(feat, w1, b1, w2, b2, out_h, out_w, trace=False, **spmd_kwargs):
    assert int(out_h) == 256 and int(out_w) == 256
    nc = get_program()
    small, mblobs, bands1, bands2 = _host_prep(feat, w1, b1, w2, b2)
    in_maps = [
        {"small": small, "band1": bands1[core], "band2": bands2[core],
         **{f"m{m}": mblobs[m] for m in range(5)}}
        for core in range(N_CORES)
    ]
    res = run_bass_kernel_spmd(
        nc, in_maps, core_ids=list(range(N_CORES)), trace=trace, **spmd_kwargs
    )
    out = _assemble([res.results[core]["out48"] for core in range(N_CORES)])
    return out, res


def kernel(feat, w1, b1, w2, b2, out_h, out_w):
    out, _ = run(feat, w1, b1, w2, b2, out_h, out_w, trace=False)
    return out


# revision 14
# speedup vs baseline: 1.1282x; 1.0466x over previous
"""MetaSR super-resolution Trainium2 kernel (bf16 v5-final).

Structure exploited: out_h=out_w=256 with H=W=64 LR grid means the scale
factor is exactly 4, so the nearest-neighbor gather index is iy=oy//4,
ix=ox//4 and the per-query MLP input collapses to 16 distinct subpixel
phases [dy/4, dx/4, 0.25].  The whole model becomes:

  1. h    = relu(mlp_in @ w1 + b1)              [16, 256]
  2. predw = h @ w2 + b2                        [16, 576, 3]
  3. rgb[o, 4*iy+dy, 4*ix+dx] =
       sum_{c,ki,kj} feat[c, iy+ki-1, ix+kj-1] * predw[(dy,dx), c*9+ki*3+kj, o]
     i.e. a 3x3 conv with 64 in / 48 out channels + pixel shuffle.

Sharding: data-parallel over LR rows (8 rows per core, 10-row halo band),
weights replicated; steps 1+2 are recomputed on every core (tiny).

The conv contraction (K = 9 taps x 64 ch = 576) is chunked K=128 by pairing
taps; each core holds the zero-padded band twice in a 128-partition tile at
free-dim offsets differing by the tap-pair shift delta (see CHUNK_SPECS).

All tensors bf16 (simulated end-to-end rel err 4.0e-3, gate 2e-2): 1-pass
PE matmuls, ~1.25MB/core input DMA.

DMA model (from NTFF traces): the 16 SDMA engines drain the two HWDGE
rings' packets in a near-serial global round-robin, so blob ARRIVAL order
follows ISSUE order across both rings and total stream time is the sum
over blobs (~280-430 GB/s).  Completion semaphores fire ~0.7us after a
blob's last row only when each engine's share (8 rows + completion desc)
fits one packet — keep rows <= 1536B.  Hence: 8 blobs <= 196KB, issued
alternating rings in exact first-use order, with conv chunks reordered
[0,2,3,1,4] so the band2-dependent chunks run last.
"""

import numpy as np
import ml_dtypes

try:
    import concourse.bass as bass
except ImportError:  # fall back to the repo checkout
    import sys
    sys.path.insert(0, "/opt/trn_rl_repo")
    import concourse.bass as bass
import concourse.mybir as mybir
import concourse.tile as tile
from concourse import bacc
from concourse.bass_utils import run_bass_kernel_spmd

F32 = mybir.dt.float32
BF16 = mybir.dt.bfloat16
N_CORES = 8
ROWS_PER_CORE = 8          # LR rows per core
BAND_ROWS = ROWS_PER_CORE + 2
NPOS = ROWS_PER_CORE * 64  # 512 LR positions per core

# Tap order for K-chunking.  Taps t = ki*3+kj have band shift ki*66+kj:
#   t:      0   1   2   3    4    5    6    7    8
#   shift:  0   1   2   66   67   68   132  133  134
# chunk0: [t0; t1] band1 off 1 | chunk1: [t3; t2] band2 off 66
# chunk2: [t4; t5] band1 off 68 | chunk3: [t6; t7] band1 off 133
# chunk4: [t8] band2 off 134 (K=64)
TAP_ORDER = [0, 1, 3, 2, 4, 5, 6, 7, 8]
CHUNK_SPECS = [  # (band_tile_idx, rhs_offset, K)
    (0, 1, 128),
    (1, 66, 128),
    (0, 68, 128),
    (0, 133, 128),
    (1, 134, 64),
]
CHUNK_ORDER = [0, 2, 3, 1, 4]  # band1 chunks first; band2 + K=64 last

# small [128, 306] bf16: b1b2 [128,17] f32 (bf16 cols 0:34),
# w1 [3,256] bf16 (rows 0-2, cols 34:290), mlpin [3,16] bf16 (cols 290:306)
OFF_W1 = 34
OFF_MLP = 290
COLS_SMALL = 306
COLS_BAND = 660   # unique band data [64, 660], shipped row-duplicated [128, 660]
COLS_BAND1 = 661
COLS_BAND2 = 724
COLS_M = 768      # w2 m-block (m<4): 6 sub-blocks (o*2+hc) x [128, 128]
COLS_M4 = 384     # m=4 block: 6 x [128, 64] (top half only)

N_WARMUP_MM = 3

_CACHE = {}


def _build_program():
    """Build + compile the single-core Bass program (same for all cores)."""
    nc = bacc.Bacc("TRN2", target_bir_lowering=False, debug=False)

    small_d = nc.dram_tensor("small", [128, COLS_SMALL], BF16, kind="ExternalInput")
    band1_d = nc.dram_tensor("band1", [128, COLS_BAND1], BF16, kind="ExternalInput")
    band2_d = nc.dram_tensor("band2", [128, COLS_BAND2], BF16, kind="ExternalInput")
    mds = [
        nc.dram_tensor(f"m{m}", [128, COLS_M if m < 4 else COLS_M4], BF16,
                       kind="ExternalInput")
        for m in range(5)
    ]
    out48 = nc.dram_tensor("out48", [48, NPOS], BF16, kind="ExternalOutput")

    with tile.TileContext(nc) as tc:
        with (
            tc.tile_pool(name="blobs", bufs=1) as blobs,
            tc.tile_pool(name="work", bufs=1) as work,
            tc.tile_pool(name="wpool", bufs=5) as wpool,
            tc.tile_pool(name="ps_small", bufs=2, space="PSUM") as ps_small,
            tc.tile_pool(name="ps_w", bufs=5, space="PSUM") as ps_w,
            tc.tile_pool(name="ps_rgb", bufs=1, space="PSUM") as ps_rgb,
        ):
            # Issue alternating rings in first-use order; the SDMA round-robin
            # then streams them in roughly this sequence.
            # Issue alternating rings in first-use order; the SDMA round-robin
            # then streams them in roughly this sequence.
            small = blobs.tile([128, COLS_SMALL], BF16, tag="small")
            nc.sync.dma_start(small[:, :], small_d[:, :])
            m0 = blobs.tile([128, COLS_M], BF16, tag="m0")
            nc.scalar.dma_start(m0[:, :], mds[0][:, :])
            m2 = blobs.tile([128, COLS_M], BF16, tag="m2")
            nc.sync.dma_start(m2[:, :], mds[2][:, :])
            band1 = blobs.tile([128, COLS_BAND1], BF16, tag="band1")
            nc.scalar.dma_start(band1[:, :], band1_d[:, :])
            m3 = blobs.tile([128, COLS_M], BF16, tag="m3")
            nc.sync.dma_start(m3[:, :], mds[3][:, :])
            m1 = blobs.tile([128, COLS_M], BF16, tag="m1")
            nc.scalar.dma_start(m1[:, :], mds[1][:, :])
            band2 = blobs.tile([128, COLS_BAND2], BF16, tag="band2")
            nc.sync.dma_start(band2[:, :], band2_d[:, :])
            m4 = blobs.tile([128, COLS_M4], BF16, tag="m4")
            nc.scalar.dma_start(m4[:, :], mds[4][:, :])

            b1b2 = small.bitcast(F32)[:, 0:17]
            w1_sb = small[0:3, OFF_W1:OFF_W1 + 256]
            mlp_sb = small[0:3, OFF_MLP:OFF_MLP + 16]
            band_tiles = [band1, band2]
            mtiles = [m0, m1, m2, m3, m4]

            def w2_slice(m, o, hc, msize):
                return mtiles[m][:, (o * 2 + hc) * msize:(o * 2 + hc + 1) * msize]

            # ---- PE warm-up: dummy zero matmuls into rgb_ps while DMAs run.
            # conv chunk 0 below uses start=True, which resets the PSUM
            # accumulation, so these contribute nothing to the result.
            rgb_ps = ps_rgb.tile([48, NPOS], F32, tag="rgb")
            warm = work.tile([128, 512], BF16, tag="warm")
            nc.vector.memset(warm[:, :], 0.0)
            for _ in range(N_WARMUP_MM):
                nc.tensor.matmul(
                    rgb_ps[:, :], warm[:, 0:48], warm[:, 0:NPOS],
                    start=True, stop=True,
                )

            # ---- MLP layer 1: h_actT [256, 16] in two 128-chunks ----
            h_sb = work.tile([128, 32], BF16, tag="hact")
            for hc in range(2):
                ph = ps_small.tile([128, 16], F32, tag="ph")
                nc.tensor.matmul(
                    ph[:, :], w1_sb[:, hc * 128:(hc + 1) * 128], mlp_sb[:, :],
                    start=True, stop=True,
                )
                # relu(x + b1) = max(x + b1, 0) in one DVE op
                nc.vector.tensor_scalar(
                    h_sb[:, hc * 16:(hc + 1) * 16], ph[:, :],
                    b1b2[:, hc:hc + 1], 0.0,
                    mybir.AluOpType.add, mybir.AluOpType.max,
                )

            # ---- per K-chunk: W assembly (MLP layer 2) + conv matmul ----
            for i, m in enumerate(CHUNK_ORDER):
                bidx, roff, K = CHUNK_SPECS[m]
                msize = K
                w_sb = wpool.tile([128, 48], BF16, tag="W")
                for o in range(3):
                    pw = ps_w.tile([128, 16], F32, tag="pw")
                    for hc in range(2):
                        nc.tensor.matmul(
                            pw[:msize, :],
                            w2_slice(m, o, hc, msize),
                            h_sb[:, hc * 16:(hc + 1) * 16],
                            start=(hc == 0), stop=(hc == 1),
                        )
                    nc.vector.tensor_scalar_add(
                        w_sb[:msize, o * 16:(o + 1) * 16], pw[:msize, :],
                        b1b2[:msize, 2 + o * 5 + m:3 + o * 5 + m],
                    )
                bt = band_tiles[bidx]
                rhs = bt[0:K, roff:roff + 528].rearrange(
                    "p (r c) -> p r c", c=66
                )[:, :, 0:64]
                nc.tensor.matmul(
                    rgb_ps[:, :], w_sb[:msize, :], rhs,
                    start=(i == 0), stop=(i == len(CHUNK_ORDER) - 1),
                )

            # ---- write out ----
            out_sb = work.tile([48, NPOS], BF16, tag="out")
            nc.vector.tensor_copy(out_sb[:, :], rgb_ps[:, :])
            nc.sync.dma_start(out48[:, :], out_sb[:, :])

    nc.compile()
    return nc


def _bf16(x):
    return np.asarray(x, dtype=np.float32).astype(ml_dtypes.bfloat16)


def _host_prep(feat, w1, b1, w2, b2):
    """Pack shared blobs + per-core band blobs (bf16)."""
    feat = np.ascontiguousarray(np.asarray(feat, dtype=np.float32))[0]  # [64,64,64]
    w1 = np.asarray(w1, dtype=np.float32)
    b1 = np.asarray(b1, dtype=np.float32)
    w2 = np.asarray(w2, dtype=np.float32)
    b2 = np.asarray(b2, dtype=np.float32)

    dydx = np.arange(16)
    mlpin = np.stack(
        [dydx // 4 / 4.0, dydx % 4 / 4.0, np.full(16, 0.25)], axis=0
    ).astype(np.float32)  # [3, 16]

    # tap-major permutations of w2/b2
    w2r = w2.reshape(256, 64, 9, 3)  # [h, c, t, o]
    w2p = np.empty((3, 256, 576), dtype=np.float32)
    b2r = b2.reshape(64, 9, 3)       # [c, t, o]
    b2p = np.empty((3, 576), dtype=np.float32)
    for blk, t in enumerate(TAP_ORDER):
        w2p[:, :, blk * 64:(blk + 1) * 64] = w2r[:, :, t, :].transpose(2, 0, 1)
        b2p[:, blk * 64:(blk + 1) * 64] = b2r[:, t, :].T
    w2p = _bf16(w2p)

    b1b2 = np.zeros((128, 17), dtype=np.float32)
    b1b2[:, 0] = b1[0:128]
    b1b2[:, 1] = b1[128:256]
    for o in range(3):
        for m in range(5):
            msize = 128 if m < 4 else 64
            b1b2[:msize, 2 + o * 5 + m] = b2p[o, 128 * m:128 * m + msize]

    small = np.zeros((128, COLS_SMALL), dtype=ml_dtypes.bfloat16)
    small.view(np.uint16)[:, 0:34] = b1b2.view(np.uint16)
    small[0:3, OFF_W1:OFF_W1 + 256] = _bf16(w1)
    small[0:3, OFF_MLP:OFF_MLP + 16] = _bf16(mlpin)

    mblobs = []
    for m in range(5):
        msize = 128 if m < 4 else 64
        blob = np.empty((128, 6 * msize), dtype=ml_dtypes.bfloat16)
        for o in range(3):
            for hc in range(2):
                base = (o * 2 + hc) * msize
                blob[:, base:base + msize] = \
                    w2p[o, hc * 128:(hc + 1) * 128, 128 * m:128 * m + msize]
        mblobs.append(blob)

    featp = np.zeros((64, 66, 66), dtype=np.float32)
    featp[:, 1:65, 1:65] = feat
    featp = _bf16(featp)

    bands1, bands2 = [], []
    for core in range(N_CORES):
        r0 = core * ROWS_PER_CORE
        band = featp[:, r0:r0 + BAND_ROWS, :].reshape(64, BAND_ROWS * 66)
        bb1 = np.zeros((128, COLS_BAND1), dtype=ml_dtypes.bfloat16)
        bb1[0:64, 1:661] = band
        bb1[64:128, 0:660] = band
        bb2 = np.zeros((128, COLS_BAND2), dtype=ml_dtypes.bfloat16)
        bb2[0:64, 0:660] = band
        bb2[64:128, 64:724] = band
        bands1.append(bb1)
        bands2.append(bb2)
    return small, mblobs, bands1, bands2


def _assemble(per_core_out48):
    """[8 x [48, 512]] -> [1, 3, 256, 256]."""
    full = np.stack([np.asarray(x, dtype=np.float32) for x in per_core_out48])
    full = full.reshape(8, 3, 4, 4, 8, 64)               # [core, o, dy, dx, r, x]
    rgb = full.transpose(1, 0, 4, 2, 5, 3).reshape(3, 256, 256)
    return np.ascontiguousarray(rgb)[None]


def get_program():
    if "nc" not in _CACHE:
        _CACHE["nc"] = _build_program()
    return _CACHE["nc"]


def run(feat, w1, b1, w2, b2, out_h, out_w, trace=False, **spmd_kwargs):
    assert int(out_h) == 256 and int(out_w) == 256
    nc = get_program()
    small, mblobs, bands1, bands2 = _host_prep(feat, w1, b1, w2, b2)
    in_maps = [
        {"small": small, "band1": bands1[core], "band2": bands2[core],
         **{f"m{m}": mblobs[m] for m in range(5)}}
        for core in range(N_CORES)
    ]
    res = run_bass_kernel_spmd(
        nc, in_maps, core_ids=list(range(N_CORES)), trace=trace, **spmd_kwargs
    )
    out = _assemble([res.results[core]["out48"] for core in range(N_CORES)])
    return out, res


def kernel(feat, w1, b1, w2, b2, out_h, out_w):
    out, _ = run(feat, w1, b1, w2, b2, out_h, out_w, trace=False)
    return out
